# revision 16
# baseline (speedup 1.0000x reference)
"""Trainium2 Bass kernel for nn_Attribution (sparse local-window attention).

Data-parallel over batch n=8 -> one batch element per NeuronCore.

Per-core computation (c_in=256, ch=128, 64x64 image):
    h    = W1 @ x + b1
    corr = 5x5 local window correlation of h (zero padded), /sqrt(128)
    attn = softmax over the 25 window entries
    samp = sum_k attn_k * shift_k(h)
    gate = sigmoid(relu(W2 @ h + b2)) = 0.5 + 0.5*relu(tanh((z+b2)/2))
    out  = Wout @ (gate * samp) + bout

Layout: positions flattened row-major with 2 zero-pad rows top/bottom
(68 rows x 64 = 4352 positions = 34 chunks of 128).  Scores are computed
"born transposed" (keys of one chunk on partitions, queries on the free
axis): for key chunk c the queries of all subs needing it are contiguous,
so one matmul (n<=384) produces all scores of that chunk.  Out-of-window
entries are killed by a {0,1} band mask after exp; out-of-image x
neighbors contribute exp(0)=1 in the zero-padded reference and are
restored by a per-partition denominator correction.

Softmax denominators are accumulated as PSUM *columns*: one matmul per
(chunk, sub) block with the exp'd score block as the stationary operand
and a ones column moving (free dim 1), all inside a single PSUM
accumulation group opened by a zeros@zeros matmul (PSUM start=True
zeroes a whole 2KB region, so interleaved per-column groups cannot
work).  The Newton-iteration reciprocal then runs directly on the
[128,32] column tile -- no DMA reshapes -- and is broadcast over
channels by rank-1 matmuls from a row gathered by one tiny DMA per
half.  Softmax normalization is commuted through the output conv.

Schedule: warmup matmuls hold the PE HAM clock gate open through the
DMA ramp; conv1, conv2/gate, and all 34 PE transposes fill the
DMA-bound startup; the score loop streams deep per-engine queues with
denominator matmuls trailing one chunk pair; the first recip chain-half
overlaps the back of the score loop, the second hides under the first
sample groups.  Output is stored bf16, all output DMAs issue on the
otherwise-idle Sync queue (keeping the ACT queue free for exp).
"""
import sys

sys.path.insert(0, "/opt/trn_rl_repo")

import numpy as np
import ml_dtypes

import concourse.bass as bass
import concourse.mybir as mybir
import concourse.tile as tile
from concourse import bacc
from concourse.bass_utils import run_bass_kernel_spmd

F32 = mybir.dt.float32
BF16 = mybir.dt.bfloat16
I32 = mybir.dt.int32
AF = mybir.ActivationFunctionType
ALU = mybir.AluOpType

N, CIN, CH, H, W = 8, 256, 128, 64, 64
HW = H * W                      # 4096
RAD = 2
KROWS = H + 2 * RAD             # 68 padded rows
PADPOS = KROWS * W              # 4352
NCHUNK = PADPOS // 128          # 34 key chunks (2 rows each)
NSUB = H // 2                   # 32 query subs (128 queries each)
NCP = NCHUNK // 2               # 17 chunk pairs
SCALE = 1.0 / np.sqrt(np.float32(CH))
RECIP_MAGIC = 0x7EF127EA


def _build_mask_and_D():
    """maskC: (128, 384) {0,1}; col 128*a+q is the score of key (chunk c,
    pos p) vs query q of sub s = c-2+a.  Valid iff |2-2a + p//64 - q//64|
    <= 2 and |p%64 - q%64| <= 2.   dcol: (128,1) = 5*cnt(qx) out-of-image
    denominator correction (identical for every sub)."""
    m = np.zeros((128, 384), dtype=np.float32)
    for a in range(3):
        for p in range(128):
            for q in range(128):
                dy = 2 - 2 * a + p // 64 - q // 64
                if abs(dy) <= RAD and abs(p % 64 - q % 64) <= RAD:
                    m[p, 128 * a + q] = 1.0
    maskC = m.astype(ml_dtypes.bfloat16)
    maskC2g = np.concatenate(
        [m, np.zeros((128, 128), np.float32), m], axis=1).astype(ml_dtypes.bfloat16)

    cnt = np.array([sum(1 for dx in range(-RAD, RAD + 1) if not 0 <= qx + dx < W)
                    for qx in range(W)], dtype=np.float32)
    dcol = (5.0 * np.concatenate([cnt, cnt])).reshape(128, 1)
    return maskC, maskC2g, dcol


def _chunk_span(c):
    """Valid sub range for key chunk c -> (lo, hi, alo, ahi)."""
    lo, hi = max(0, c - 2), min(NSUB - 1, c)
    return lo, hi, lo - (c - 2), hi - (c - 2)


def build_nc(repeat=1, sim_safe=False):
    nc = bacc.Bacc("TRN2", target_bir_lowering=False, debug=False, num_devices=8)

    x_d = nc.declare_dram_parameter("x", [CIN, HW], BF16, isOutput=False)
    w1t_d = nc.declare_dram_parameter("W1T", [CIN, CH], BF16, isOutput=False)
    b1_d = nc.declare_dram_parameter("b1", [CH, 1], F32, isOutput=False)
    w2t_d = nc.declare_dram_parameter("W2T", [CH, CH], BF16, isOutput=False)
    b2h_d = nc.declare_dram_parameter("b2h", [CH, 1], F32, isOutput=False)
    wot_d = nc.declare_dram_parameter("WoutT", [CH, CIN], BF16, isOutput=False)
    bout_d = nc.declare_dram_parameter("bout2", [CH, 2], F32, isOutput=False)
    boutr_d = nc.declare_dram_parameter("boutrow", [1, CIN], BF16, isOutput=False)
    mask_d = nc.declare_dram_parameter("maskC", [128, 384], BF16, isOutput=False)
    mask2_d = nc.declare_dram_parameter("maskC2g", [128, 896], BF16, isOutput=False)
    dcol_d = nc.declare_dram_parameter("dcol", [128, 1], F32, isOutput=False)
    ident_d = nc.declare_dram_parameter("ident", [128, 128], BF16, isOutput=False)
    onescol_d = nc.declare_dram_parameter("onescol_c", [128, 1], BF16, isOutput=False)
    ones1_d = nc.declare_dram_parameter("ones1_c", [1, 512], BF16, isOutput=False)
    out_d = nc.declare_dram_parameter("out", [CIN, HW], BF16, isOutput=True)

    with tile.TileContext(nc) as tc:
        with (
            tc.tile_pool(name="per", bufs=1) as per,
            tc.tile_pool(name="xb", bufs=4) as xbp,
            tc.tile_pool(name="sm", bufs=4) as smp,
            tc.tile_pool(name="ot", bufs=4) as otp,
            tc.tile_pool(name="psS", bufs=2, space="PSUM") as psS,   # 2 banks each
            tc.tile_pool(name="psA", bufs=3, space="PSUM") as psA,   # 1 bank each
            tc.tile_pool(name="psD", bufs=1, space="PSUM") as psD,   # denominators
        ):
            hpad = per.tile([128, PADPOS], BF16, tag="hpad")
            hT = per.tile([128, PADPOS], BF16, tag="hT")
            attnm = per.tile([128, NCHUNK * 512], BF16, tag="attnm")
            Pg = per.tile([128, HW], BF16, tag="Pg")
            attr = per.tile([128, HW], BF16, tag="attr")

            w1t0 = per.tile([128, CH], BF16, tag="w1t0")
            w1t1 = per.tile([128, CH], BF16, tag="w1t1")
            w2t = per.tile([128, CH], BF16, tag="w2t")
            wot = per.tile([128, CIN], BF16, tag="wot")
            b1 = per.tile([CH, 1], F32, tag="b1")
            b2h = per.tile([CH, 1], F32, tag="b2h")
            bout = per.tile([CH, 2], F32, tag="bout")
            boutrow = per.tile([1, CIN], BF16, tag="boutrow")
            maskC = per.tile([128, 384], BF16, tag="maskC")
            maskC2g = per.tile([128, 896], BF16, tag="maskC2g")
            dcol = per.tile([128, 1], F32, tag="dcol")
            onescol = per.tile([128, 1], BF16, tag="onescol")
            ones1 = per.tile([1, 512], BF16, tag="ones1")
            ident = per.tile([128, 128], BF16, tag="ident")
            wtile = per.tile([128, 512], BF16, tag="wtile")
            newt = per.tile([128, 32], F32, tag="newt")
            ntmp = per.tile([128, 32], F32, tag="ntmp")
            dS = per.tile([128, 32], F32, tag="dS")
            recipS = per.tile([128, 32], BF16, tag="recipS")
            recipT = per.tile([16, 256], BF16, tag="recipT")
            recRow = per.tile([1, HW], BF16, tag="recRow")

            for _rep in range(repeat):
                # ---- startup: memsets + DMA issue (x first), warmup MMs
                nc.vector.memset(wtile[:], 0.0)
                nc.vector.memset(hpad[:, 0:128], 0.0)
                nc.vector.memset(hpad[:, PADPOS - 128:PADPOS], 0.0)
                xts = []
                for t in range(4):
                    x0 = xbp.tile([128, 1024], BF16, tag="x0")
                    x1 = xbp.tile([128, 1024], BF16, tag="x1")
                    cs = slice(1024 * t, 1024 * (t + 1))
                    nc.sync.dma_start(x0[:], x_d[0:128, cs])
                    nc.sync.dma_start(x1[:], x_d[128:256, cs])
                    xts.append((x0, x1))
                nc.sync.dma_start(w1t0[:], w1t_d[0:128, :])
                nc.sync.dma_start(w1t1[:], w1t_d[128:256, :])
                nc.sync.dma_start(b1[:], b1_d[:])
                nc.sync.dma_start(ident[:], ident_d[:])
                nc.sync.dma_start(w2t[:], w2t_d[:])
                nc.sync.dma_start(b2h[:], b2h_d[:])
                nc.sync.dma_start(maskC[:], mask_d[:])
                nc.sync.dma_start(maskC2g[:], mask2_d[:])
                nc.sync.dma_start(onescol[:], onescol_d[:])
                nc.sync.dma_start(ones1[:], ones1_d[:])
                nc.sync.dma_start(dcol[:], dcol_d[:])
                nc.sync.dma_start(wot[:], wot_d[:])
                nc.sync.dma_start(bout[:], bout_d[:])
                nc.sync.dma_start(boutrow[:], boutr_d[:])

                # PE warmup: hold the HAM clock gate open through the DMA ramp
                pw = psA.tile([128, 512], F32, tag="psa", name="warm")
                for _ in range(8):
                    nc.tensor.matmul(pw[:], wtile[:, 0:128], wtile[:],
                                     start=True, stop=True)

                # ---- P1: conv1 (PE) + bias evac (DVE) + PE transposes -> hT
                def transpose_batch(chunks):
                    # [128,1024] bf16 == 2KB/partition, same slot size as the
                    # pool's f32 [128,512] tiles (tag requires equal bytes)
                    pt = psA.tile([128, 1024], BF16, tag="psa",
                                  name=f"pt{chunks[0]}")
                    for k, c in enumerate(chunks):
                        nc.tensor.transpose(pt[:, 128 * k:128 * (k + 1)],
                                            hpad[:, 128 * c:128 * (c + 1)],
                                            ident[:])
                    nc.vector.tensor_copy(
                        hT[:, 128 * chunks[0]:128 * (chunks[0] + len(chunks))],
                        pt[:, 0:128 * len(chunks)])

                def emit_conv2(t):
                    pz = psA.tile([128, 512], F32, tag="psa", name=f"pz{t}")
                    nc.tensor.matmul(pz[:], w2t[:],
                                     hpad[:, 128 + 512 * t:128 + 512 * (t + 1)],
                                     start=True, stop=True)
                    tg = smp.tile([128, 512], BF16, tag="tg")
                    nc.scalar.activation(tg[:], pz[:], AF.Tanh, scale=0.5,
                                         bias=b2h[:])
                    nc.gpsimd.tensor_scalar(
                        out=Pg[:, 512 * t:512 * (t + 1)], in0=tg[:],
                        scalar1=0.0, scalar2=1.0, op0=ALU.max, op1=ALU.add)

                # ---- denominator accumulator: one PSUM group for the whole
                # [128,32] tile (start=True zeroes a full 2KB region, so open
                # the group once with zeros@zeros and accumulate start=False).
                denqP = psD.tile([128, 32], F32, tag="denq")
                nc.tensor.matmul(denqP[:], wtile[:, 0:128], wtile[:, 0:32],
                                 start=True, stop=False, skip_group_check=True)

                def emit_dn(c):
                    lo, hi, _, _ = _chunk_span(c)
                    for s in range(lo, hi + 1):
                        aa = s - c + 2
                        nc.tensor.matmul(
                            denqP[:, s:s + 1],
                            attnm[:, 512 * c + 128 * aa:512 * c + 128 * (aa + 1)],
                            onescol[:],
                            start=False, stop=(c == 33 and s == 31),
                            skip_group_check=True)

                def emit_scores(cp):
                    sc = psS.tile([128, 1024], F32, tag="psc", name=f"sc{cp}")
                    spans = []
                    for ci in range(2):
                        c = 2 * cp + ci
                        lo, hi, alo, ahi = _chunk_span(c)
                        spans.append((alo, ahi + 1))
                        dst = sc[:, 512 * ci + 128 * alo:512 * ci + 128 * (ahi + 1)]
                        nc.tensor.matmul(
                            dst, hpad[:, 128 * c:128 * (c + 1)],
                            hpad[:, 128 * (lo + 1):128 * (hi + 2)],
                            start=True, stop=True)
                    if not sim_safe and spans == [(0, 3), (0, 3)]:
                        asl = attnm[:, 1024 * cp:1024 * cp + 896]
                        nc.scalar.activation(asl, sc[:, 0:896], AF.Exp,
                                             scale=float(SCALE))
                        eng = nc.gpsimd if cp % 2 == 0 else nc.vector
                        eng.tensor_tensor(out=asl, in0=asl,
                                          in1=maskC2g[:], op=ALU.mult)
                    else:
                        for ci, (a0, a1) in enumerate(spans):
                            ss = slice(512 * ci + 128 * a0, 512 * ci + 128 * a1)
                            asl = attnm[:, 1024 * cp + ss.start:1024 * cp + ss.stop]
                            nc.scalar.activation(asl, sc[:, ss], AF.Exp,
                                                 scale=float(SCALE))
                            nc.vector.tensor_tensor(
                                out=asl, in0=asl,
                                in1=maskC[:, 128 * a0:128 * a1], op=ALU.mult)

                def emit_chain_newton(h):
                    """Newton recip on denominator half h (subs 16h..16h+15)."""
                    qs = slice(16 * h, 16 * (h + 1))
                    nc.vector.tensor_scalar(out=dS[:, qs], in0=denqP[:, qs],
                                            scalar1=dcol[:], scalar2=None,
                                            op0=ALU.add)
                    nc.vector.tensor_scalar(out=newt[:, qs].bitcast(I32),
                                            in0=dS[:, qs].bitcast(I32),
                                            scalar1=0, scalar2=None,
                                            op0=ALU.bitwise_not)
                    nc.vector.tensor_scalar(out=newt[:, qs].bitcast(I32),
                                            in0=newt[:, qs].bitcast(I32),
                                            scalar1=RECIP_MAGIC + 1,
                                            scalar2=None, op0=ALU.add)
                    for _ in range(3):
                        nc.vector.tensor_tensor(out=ntmp[:, qs], in0=dS[:, qs],
                                                in1=newt[:, qs], op=ALU.mult)
                        nc.vector.tensor_scalar(out=ntmp[:, qs], in0=ntmp[:, qs],
                                                scalar1=-1.0, scalar2=2.0,
                                                op0=ALU.mult, op1=ALU.add)
                        nc.vector.tensor_tensor(out=newt[:, qs], in0=newt[:, qs],
                                                in1=ntmp[:, qs], op=ALU.mult)
                    nc.vector.tensor_scalar(out=recipS[:, qs], in0=newt[:, qs],
                                            scalar1=0.5, scalar2=None,
                                            op0=ALU.mult)

                def emit_chain_row(h):
                    """recip columns -> rows (PE transpose) + tiny row gather."""
                    qs = slice(16 * h, 16 * (h + 1))
                    ptr = psA.tile([128, 1024], BF16, tag="psa", name=f"ptr{h}")
                    nc.tensor.transpose(ptr[0:16, 0:128], recipS[:, qs], ident[:])
                    nc.vector.tensor_copy(recipT[0:16, 128 * h:128 * (h + 1)],
                                          ptr[0:16, 0:128])
                    nc.sync.dma_start(
                        recRow[0:1, 2048 * h:2048 * (h + 1)].rearrange(
                            "o (s f) -> o s f", s=16),
                        recipT[0:16, 128 * h:128 * (h + 1)])

                def emit_pb_pgs(g8):
                    pb = psA.tile([128, 512], F32, tag="psa", name=f"pb{g8}")
                    nc.tensor.matmul(pb[:], ones1[0:1, 0:128],
                                     recRow[0:1, 512 * g8:512 * (g8 + 1)],
                                     start=True, stop=True)
                    gsl = slice(512 * g8, 512 * (g8 + 1))
                    nc.vector.tensor_tensor(out=Pg[:, gsl], in0=Pg[:, gsl],
                                            in1=pb[:], op=ALU.mult)

                # ---- fused ramp: each x tile t unlocks conv1 tile t, score
                # chunk-pairs 4t..4t+3 (keys AND query spans stay inside the
                # tile), the matching conv2 tiles and transposes, and the
                # previous tile's denominator matmuls.  The whole score loop
                # hides in the DMA shadow; ACT streams exp continuously.
                tr_batches = {
                    0: ([0, 1, 2, 3], [4, 5, 6, 7]),
                    1: ([8, 9, 10, 11], [12, 13, 14, 15]),
                    2: ([16, 17, 18, 19], [20, 21, 22, 23]),
                    3: ([24, 25, 26, 27], [28, 29, 30, 31], [32, 33]),
                }
                for t in range(4):
                    x0, x1 = xts[t]
                    for u in range(2):
                        pc = psA.tile([128, 512], F32, tag="psa", name=f"c1_{t}{u}")
                        usl = slice(512 * u, 512 * (u + 1))
                        nc.tensor.matmul(pc[:], w1t0[:], x0[:, usl],
                                         start=True, stop=False)
                        nc.tensor.matmul(pc[:], w1t1[:], x1[:, usl],
                                         start=False, stop=True)
                        o = 128 + 1024 * t + 512 * u
                        nc.vector.tensor_scalar(
                            out=hpad[:, o:o + 512], in0=pc[:],
                            scalar1=b1[:], scalar2=None, op0=ALU.add)
                    if t == 3:
                        for p in range(8, 12):
                            emit_dn(2 * p)
                            emit_dn(2 * p + 1)
                        emit_chain_newton(0)
                    for cp in range(4 * t, 4 * t + 4):
                        emit_scores(cp)
                    if t == 3:
                        emit_scores(16)
                        emit_chain_row(0)
                    emit_conv2(2 * t)
                    emit_conv2(2 * t + 1)
                    for batch in tr_batches[t]:
                        transpose_batch(batch)
                    if 1 <= t <= 2:
                        for p in range(4 * (t - 1), 4 * t):
                            emit_dn(2 * p)
                            emit_dn(2 * p + 1)

                for p in range(12, 16):
                    emit_dn(2 * p)
                    emit_dn(2 * p + 1)

                # ---- P3e: samples (pairs of g8 share one 2-bank psum tile),
                # gate+normalize, output conv; chain half 1 hides under the
                # first sample pair.
                sp_tiles = {}

                def emit_sample_pair(gp):
                    sp = psS.tile([128, 1024], F32, tag="psc", name=f"sp{gp}")
                    sp_tiles[gp] = sp
                    for a8 in range(8):
                        s8 = 8 * gp + a8
                        for j in range(3):
                            c = s8 + j
                            aa = 2 - j
                            nc.tensor.matmul(
                                sp[:, 128 * a8:128 * (a8 + 1)],
                                hT[:, 128 * c:128 * (c + 1)],
                                attnm[:, 512 * c + 128 * aa:512 * c + 128 * (aa + 1)],
                                start=(j == 0), stop=(j == 2))

                def emit_attr(g8):
                    sp = sp_tiles[g8 // 2]
                    gsl = slice(512 * g8, 512 * (g8 + 1))
                    nc.vector.tensor_tensor(
                        out=attr[:, gsl], in0=sp[:, 512 * (g8 % 2):512 * (g8 % 2 + 1)],
                        in1=Pg[:, gsl], op=ALU.mult)

                def emit_convout(g8):
                    gsl = slice(512 * g8, 512 * (g8 + 1))
                    for oc in range(2):
                        po = psA.tile([128, 512], F32, tag="psa",
                                      name=f"po{g8}_{oc}")
                        if oc == 1:
                            nc.tensor.matmul(po[:], boutrow[0:1, 128:256],
                                             ones1[0:1, :], start=True, stop=False)
                            nc.tensor.matmul(po[:], wot[:, 128:256], attr[:, gsl],
                                             start=False, stop=True)
                        else:
                            nc.tensor.matmul(po[:], wot[:, 0:128], attr[:, gsl],
                                             start=True, stop=True)
                        ot = otp.tile([128, 512], BF16, tag="ot")
                        if oc == 1:
                            nc.scalar.activation(ot[:], po[:], AF.Copy)
                        else:
                            nc.vector.tensor_scalar(out=ot[:], in0=po[:],
                                                    scalar1=bout[:, 0:1],
                                                    scalar2=None, op0=ALU.add)
                        nc.sync.dma_start(out_d[128 * oc:128 * (oc + 1), gsl],
                                          ot[:])

                emit_sample_pair(0)
                emit_pb_pgs(0)
                emit_pb_pgs(1)
                emit_dn(32)
                emit_dn(33)
                emit_chain_newton(1)
                emit_sample_pair(1)
                emit_chain_row(1)
                emit_attr(0)
                emit_convout(0)
                emit_attr(1)
                emit_convout(1)
                emit_pb_pgs(2)
                emit_pb_pgs(3)
                emit_sample_pair(2)
                emit_attr(2)
                emit_convout(2)
                emit_attr(3)
                emit_convout(3)
                emit_pb_pgs(4)
                emit_pb_pgs(5)
                emit_sample_pair(3)
                emit_attr(4)
                emit_convout(4)
                emit_attr(5)
                emit_convout(5)
                emit_pb_pgs(6)
                emit_pb_pgs(7)
                emit_attr(6)
                emit_convout(6)
                emit_attr(7)
                emit_convout(7)

    return nc


def _prep_inputs(x, W1, b1, W2, b2, Wout, bout):
    maskC, maskC2g, dcol = _build_mask_and_D()
    bf = ml_dtypes.bfloat16
    common = {
        "W1T": np.ascontiguousarray(W1.T).astype(bf),
        "b1": np.asarray(b1, np.float32).reshape(CH, 1),
        "W2T": np.ascontiguousarray(W2.T).astype(bf),
        "b2h": (0.5 * np.asarray(b2, np.float32)).reshape(CH, 1),
        "WoutT": np.ascontiguousarray(Wout.T).astype(bf),
        "bout2": np.ascontiguousarray(np.asarray(bout, np.float32).reshape(2, CH).T),
        "boutrow": np.asarray(bout, np.float32).reshape(1, CIN).astype(bf),
        "maskC": maskC,
        "maskC2g": maskC2g,
        "dcol": dcol,
        "ident": np.eye(128, dtype=np.float32).astype(bf),
        "onescol_c": np.ones((128, 1), np.float32).astype(bf),
        "ones1_c": np.ones((1, 512), np.float32).astype(bf),
    }
    in_maps = []
    for i in range(N):
        m = dict(common)
        m["x"] = np.ascontiguousarray(
            np.asarray(x[i], np.float32).reshape(CIN, HW)).astype(bf)
        in_maps.append(m)
    return in_maps


_CACHED = {}


def kernel(x, W1, b1, W2, b2, Wout, bout):
    if "nc" not in _CACHED:
        nc = build_nc()
        nc.finalize()
        _CACHED["nc"] = nc
    nc = _CACHED["nc"]
    in_maps = _prep_inputs(x, W1, b1, W2, b2, Wout, bout)
    res = run_bass_kernel_spmd(nc, in_maps, core_ids=list(range(N)))
    out = np.stack([res.results[i]["out"].reshape(CIN, H, W) for i in range(N)])
    return out.astype(np.float32)


# revision 17
# speedup vs baseline: 1.6700x; 1.6700x over previous
"""Trainium2 Bass kernel for nn_Attribution (sparse local-window attention).

Data-parallel over batch n=8 -> one batch element per NeuronCore.

Per-core computation (c_in=256, ch=128, 64x64 image):
    h    = W1 @ x + b1
    corr = 5x5 local window correlation of h (zero padded), /sqrt(128)
    attn = softmax over the 25 window entries
    samp = sum_k attn_k * shift_k(h)
    gate = sigmoid(relu(W2 @ h + b2)) = 0.5 + 0.5*relu(tanh((z+b2)/2))
    out  = Wout @ (gate * samp) + bout

Layout: positions flattened row-major with 2 zero-pad rows top/bottom
(68 rows x 64 = 4352 positions = 34 chunks of 128).  Scores are computed
"born transposed" (keys of one chunk on partitions, queries on the free
axis): for key chunk c the queries of all subs needing it are contiguous,
so one matmul (n<=384) produces all scores of that chunk.  Out-of-window
entries are killed by a {0,1} band mask after exp; out-of-image x
neighbors contribute exp(0)=1 in the zero-padded reference and are
restored by a per-partition denominator correction.

Softmax denominators are accumulated as PSUM *columns*: one matmul per
(chunk, sub) block with the exp'd score block as the stationary operand
and a ones column moving (free dim 1), all inside a single PSUM
accumulation group opened by a zeros@zeros matmul (PSUM start=True
zeroes a whole 2KB region, so interleaved per-column groups cannot
work).  The Newton-iteration reciprocal then runs directly on the
[128,32] column tile -- no DMA reshapes -- and is broadcast over
channels by rank-1 matmuls from a row gathered by one tiny DMA per
half.  Softmax normalization is commuted through the output conv.

Schedule: warmup matmuls hold the PE HAM clock gate open through the
DMA ramp; conv1, conv2/gate, and all 34 PE transposes fill the
DMA-bound startup; the score loop streams deep per-engine queues with
denominator matmuls trailing one chunk pair; the first recip chain-half
overlaps the back of the score loop, the second hides under the first
sample groups.  Output is stored bf16, all output DMAs issue on the
otherwise-idle Sync queue (keeping the ACT queue free for exp).
"""
import sys

sys.path.insert(0, "/opt/trn_rl_repo")

import numpy as np
import ml_dtypes

import concourse.bass as bass
import concourse.mybir as mybir
import concourse.tile as tile
from concourse import bacc
from concourse.bass_utils import run_bass_kernel_spmd

F32 = mybir.dt.float32
BF16 = mybir.dt.bfloat16
I32 = mybir.dt.int32
AF = mybir.ActivationFunctionType
ALU = mybir.AluOpType

N, CIN, CH, H, W = 8, 256, 128, 64, 64
HW = H * W                      # 4096
RAD = 2
KROWS = H + 2 * RAD             # 68 padded rows
PADPOS = KROWS * W              # 4352
NCHUNK = PADPOS // 128          # 34 key chunks (2 rows each)
NSUB = H // 2                   # 32 query subs (128 queries each)
NCP = NCHUNK // 2               # 17 chunk pairs
SCALE = 1.0 / np.sqrt(np.float32(CH))
RECIP_MAGIC = 0x7EF127EA


def _build_mask_and_D():
    """maskC: (128, 384) {0,1}; col 128*a+q is the score of key (chunk c,
    pos p) vs query q of sub s = c-2+a.  Valid iff |2-2a + p//64 - q//64|
    <= 2 and |p%64 - q%64| <= 2.   dcol: (128,1) = 5*cnt(qx) out-of-image
    denominator correction (identical for every sub)."""
    m = np.zeros((128, 384), dtype=np.float32)
    for a in range(3):
        for p in range(128):
            for q in range(128):
                dy = 2 - 2 * a + p // 64 - q // 64
                if abs(dy) <= RAD and abs(p % 64 - q % 64) <= RAD:
                    m[p, 128 * a + q] = 1.0
    maskC = m.astype(ml_dtypes.bfloat16)
    maskC2g = np.concatenate(
        [m, np.zeros((128, 128), np.float32), m], axis=1).astype(ml_dtypes.bfloat16)

    cnt = np.array([sum(1 for dx in range(-RAD, RAD + 1) if not 0 <= qx + dx < W)
                    for qx in range(W)], dtype=np.float32)
    dcol = (5.0 * np.concatenate([cnt, cnt])).reshape(128, 1)
    return maskC, maskC2g, dcol


def _chunk_span(c):
    """Valid sub range for key chunk c -> (lo, hi, alo, ahi)."""
    lo, hi = max(0, c - 2), min(NSUB - 1, c)
    return lo, hi, lo - (c - 2), hi - (c - 2)


def build_nc(repeat=1, sim_safe=False):
    nc = bacc.Bacc("TRN2", target_bir_lowering=False, debug=False, num_devices=8)

    x_d = nc.declare_dram_parameter("x", [CIN, HW], BF16, isOutput=False)
    w1t_d = nc.declare_dram_parameter("W1T", [CIN, CH], BF16, isOutput=False)
    b1_d = nc.declare_dram_parameter("b1", [CH, 1], F32, isOutput=False)
    w2t_d = nc.declare_dram_parameter("W2T", [CH, CH], BF16, isOutput=False)
    b2h_d = nc.declare_dram_parameter("b2h", [CH, 1], F32, isOutput=False)
    wot_d = nc.declare_dram_parameter("WoutT", [CH, CIN], BF16, isOutput=False)
    bout_d = nc.declare_dram_parameter("bout2", [CH, 2], F32, isOutput=False)
    boutr_d = nc.declare_dram_parameter("boutrow", [1, CIN], BF16, isOutput=False)
    mask_d = nc.declare_dram_parameter("maskC", [128, 384], BF16, isOutput=False)
    mask2_d = nc.declare_dram_parameter("maskC2g", [128, 896], BF16, isOutput=False)
    dcol_d = nc.declare_dram_parameter("dcol", [128, 1], F32, isOutput=False)
    ident_d = nc.declare_dram_parameter("ident", [128, 128], BF16, isOutput=False)
    onescol_d = nc.declare_dram_parameter("onescol_c", [128, 1], BF16, isOutput=False)
    ones1_d = nc.declare_dram_parameter("ones1_c", [1, 512], BF16, isOutput=False)
    out_d = nc.declare_dram_parameter("out", [CIN, HW], BF16, isOutput=True)

    with tile.TileContext(nc) as tc:
        with (
            tc.tile_pool(name="per", bufs=1) as per,
            tc.tile_pool(name="xb", bufs=4) as xbp,
            tc.tile_pool(name="sm", bufs=4) as smp,
            tc.tile_pool(name="ot", bufs=4) as otp,
            tc.tile_pool(name="psS", bufs=2, space="PSUM") as psS,   # 2 banks each
            tc.tile_pool(name="psA", bufs=3, space="PSUM") as psA,   # 1 bank each
            tc.tile_pool(name="psD", bufs=1, space="PSUM") as psD,   # denominators
        ):
            hpad = per.tile([128, PADPOS], BF16, tag="hpad")
            hT = per.tile([128, PADPOS], BF16, tag="hT")
            attnm = per.tile([128, NCHUNK * 512], BF16, tag="attnm")
            Pg = per.tile([128, HW], BF16, tag="Pg")
            attr = per.tile([128, HW], BF16, tag="attr")

            w1t0 = per.tile([128, CH], BF16, tag="w1t0")
            w1t1 = per.tile([128, CH], BF16, tag="w1t1")
            w2t = per.tile([128, CH], BF16, tag="w2t")
            wot = per.tile([128, CIN], BF16, tag="wot")
            b1 = per.tile([CH, 1], F32, tag="b1")
            b2h = per.tile([CH, 1], F32, tag="b2h")
            bout = per.tile([CH, 2], F32, tag="bout")
            boutrow = per.tile([1, CIN], BF16, tag="boutrow")
            maskC = per.tile([128, 384], BF16, tag="maskC")
            maskC2g = per.tile([128, 896], BF16, tag="maskC2g")
            dcol = per.tile([128, 1], F32, tag="dcol")
            onescol = per.tile([128, 1], BF16, tag="onescol")
            ones1 = per.tile([1, 512], BF16, tag="ones1")
            ident = per.tile([128, 128], BF16, tag="ident")
            wtile = per.tile([128, 512], BF16, tag="wtile")
            newt = per.tile([128, 32], F32, tag="newt")
            ntmp = per.tile([128, 32], F32, tag="ntmp")
            dS = per.tile([128, 32], F32, tag="dS")
            recipS = per.tile([128, 32], BF16, tag="recipS")
            recipT = per.tile([16, 256], BF16, tag="recipT")
            recRow = per.tile([1, HW], BF16, tag="recRow")

            for _rep in range(repeat):
                # ---- startup: memsets + DMA issue (x first), warmup MMs
                nc.vector.memset(wtile[:], 0.0)
                nc.vector.memset(hpad[:, 0:128], 0.0)
                nc.vector.memset(hpad[:, PADPOS - 128:PADPOS], 0.0)
                xts = []
                for t in range(4):
                    x0 = xbp.tile([128, 1024], BF16, tag="x0")
                    x1 = xbp.tile([128, 1024], BF16, tag="x1")
                    cs = slice(1024 * t, 1024 * (t + 1))
                    nc.sync.dma_start(x0[:], x_d[0:128, cs])
                    nc.scalar.dma_start(x1[:], x_d[128:256, cs])
                    xts.append((x0, x1))
                nc.sync.dma_start(w1t0[:], w1t_d[0:128, :])
                nc.sync.dma_start(w1t1[:], w1t_d[128:256, :])
                nc.sync.dma_start(b1[:], b1_d[:])
                nc.sync.dma_start(ident[:], ident_d[:])
                nc.sync.dma_start(w2t[:], w2t_d[:])
                nc.sync.dma_start(b2h[:], b2h_d[:])
                nc.sync.dma_start(maskC[:], mask_d[:])
                nc.sync.dma_start(maskC2g[:], mask2_d[:])
                nc.sync.dma_start(onescol[:], onescol_d[:])
                nc.sync.dma_start(ones1[:], ones1_d[:])
                nc.sync.dma_start(dcol[:], dcol_d[:])
                nc.sync.dma_start(wot[:], wot_d[:])
                nc.sync.dma_start(bout[:], bout_d[:])
                nc.sync.dma_start(boutrow[:], boutr_d[:])

                # PE warmup: hold the HAM clock gate open through the DMA ramp
                pw = psA.tile([128, 512], F32, tag="psa", name="warm")
                for _ in range(8):
                    nc.tensor.matmul(pw[:], wtile[:, 0:128], wtile[:],
                                     start=True, stop=True)

                # ---- P1: conv1 (PE) + bias evac (DVE) + PE transposes -> hT
                def transpose_batch(chunks):
                    # [128,1024] bf16 == 2KB/partition, same slot size as the
                    # pool's f32 [128,512] tiles (tag requires equal bytes)
                    pt = psA.tile([128, 1024], BF16, tag="psa",
                                  name=f"pt{chunks[0]}")
                    for k, c in enumerate(chunks):
                        nc.tensor.transpose(pt[:, 128 * k:128 * (k + 1)],
                                            hpad[:, 128 * c:128 * (c + 1)],
                                            ident[:])
                    nc.vector.tensor_copy(
                        hT[:, 128 * chunks[0]:128 * (chunks[0] + len(chunks))],
                        pt[:, 0:128 * len(chunks)])

                def emit_conv2(t):
                    pz = psA.tile([128, 512], F32, tag="psa", name=f"pz{t}")
                    nc.tensor.matmul(pz[:], w2t[:],
                                     hpad[:, 128 + 512 * t:128 + 512 * (t + 1)],
                                     start=True, stop=True)
                    tg = smp.tile([128, 512], BF16, tag="tg")
                    nc.scalar.activation(tg[:], pz[:], AF.Tanh, scale=0.5,
                                         bias=b2h[:])
                    nc.vector.tensor_scalar(
                        out=Pg[:, 512 * t:512 * (t + 1)], in0=tg[:],
                        scalar1=0.0, scalar2=1.0, op0=ALU.max, op1=ALU.add)

                # ---- denominator accumulator: one PSUM group for the whole
                # [128,32] tile (start=True zeroes a full 2KB region, so open
                # the group once with zeros@zeros and accumulate start=False).
                denqP = psD.tile([128, 32], F32, tag="denq")
                nc.tensor.matmul(denqP[:], wtile[:, 0:128], wtile[:, 0:32],
                                 start=True, stop=False, skip_group_check=True)

                def emit_dn(c):
                    lo, hi, _, _ = _chunk_span(c)
                    for s in range(lo, hi + 1):
                        aa = s - c + 2
                        nc.tensor.matmul(
                            denqP[:, s:s + 1],
                            attnm[:, 512 * c + 128 * aa:512 * c + 128 * (aa + 1)],
                            onescol[:],
                            start=False, stop=(c == 33 and s == 31),
                            skip_group_check=True)

                def emit_scores(cp):
                    sc = psS.tile([128, 1024], F32, tag="psc", name=f"sc{cp}")
                    spans = []
                    for ci in range(2):
                        c = 2 * cp + ci
                        lo, hi, alo, ahi = _chunk_span(c)
                        spans.append((alo, ahi + 1))
                        dst = sc[:, 512 * ci + 128 * alo:512 * ci + 128 * (ahi + 1)]
                        nc.tensor.matmul(
                            dst, hpad[:, 128 * c:128 * (c + 1)],
                            hpad[:, 128 * (lo + 1):128 * (hi + 2)],
                            start=True, stop=True)
                    if not sim_safe and spans == [(0, 3), (0, 3)]:
                        asl = attnm[:, 1024 * cp:1024 * cp + 896]
                        nc.scalar.activation(asl, sc[:, 0:896], AF.Exp,
                                             scale=float(SCALE))
                        nc.vector.tensor_tensor(out=asl, in0=asl,
                                                in1=maskC2g[:], op=ALU.mult)
                    else:
                        for ci, (a0, a1) in enumerate(spans):
                            ss = slice(512 * ci + 128 * a0, 512 * ci + 128 * a1)
                            asl = attnm[:, 1024 * cp + ss.start:1024 * cp + ss.stop]
                            nc.scalar.activation(asl, sc[:, ss], AF.Exp,
                                                 scale=float(SCALE))
                            nc.vector.tensor_tensor(
                                out=asl, in0=asl,
                                in1=maskC[:, 128 * a0:128 * a1], op=ALU.mult)

                def emit_chain_newton(h):
                    """Newton recip on denominator half h (subs 16h..16h+15)."""
                    qs = slice(16 * h, 16 * (h + 1))
                    nc.vector.tensor_scalar(out=dS[:, qs], in0=denqP[:, qs],
                                            scalar1=dcol[:], scalar2=None,
                                            op0=ALU.add)
                    nc.vector.tensor_scalar(out=newt[:, qs].bitcast(I32),
                                            in0=dS[:, qs].bitcast(I32),
                                            scalar1=0, scalar2=None,
                                            op0=ALU.bitwise_not)
                    nc.vector.tensor_scalar(out=newt[:, qs].bitcast(I32),
                                            in0=newt[:, qs].bitcast(I32),
                                            scalar1=RECIP_MAGIC + 1,
                                            scalar2=None, op0=ALU.add)
                    for _ in range(3):
                        nc.vector.tensor_tensor(out=ntmp[:, qs], in0=dS[:, qs],
                                                in1=newt[:, qs], op=ALU.mult)
                        nc.vector.tensor_scalar(out=ntmp[:, qs], in0=ntmp[:, qs],
                                                scalar1=-1.0, scalar2=2.0,
                                                op0=ALU.mult, op1=ALU.add)
                        nc.vector.tensor_tensor(out=newt[:, qs], in0=newt[:, qs],
                                                in1=ntmp[:, qs], op=ALU.mult)
                    nc.vector.tensor_scalar(out=recipS[:, qs], in0=newt[:, qs],
                                            scalar1=0.5, scalar2=None,
                                            op0=ALU.mult)

                def emit_chain_row(h):
                    """recip columns -> rows (PE transpose) + tiny row gather."""
                    qs = slice(16 * h, 16 * (h + 1))
                    ptr = psA.tile([128, 1024], BF16, tag="psa", name=f"ptr{h}")
                    nc.tensor.transpose(ptr[0:16, 0:128], recipS[:, qs], ident[:])
                    nc.vector.tensor_copy(recipT[0:16, 128 * h:128 * (h + 1)],
                                          ptr[0:16, 0:128])
                    nc.sync.dma_start(
                        recRow[0:1, 2048 * h:2048 * (h + 1)].rearrange(
                            "o (s f) -> o s f", s=16),
                        recipT[0:16, 128 * h:128 * (h + 1)])

                def emit_pb_pgs(g8):
                    pb = psA.tile([128, 512], F32, tag="psa", name=f"pb{g8}")
                    nc.tensor.matmul(pb[:], ones1[0:1, 0:128],
                                     recRow[0:1, 512 * g8:512 * (g8 + 1)],
                                     start=True, stop=True)
                    gsl = slice(512 * g8, 512 * (g8 + 1))
                    nc.vector.tensor_tensor(out=Pg[:, gsl], in0=Pg[:, gsl],
                                            in1=pb[:], op=ALU.mult)

                # ---- fused ramp: each x tile t unlocks conv1 tile t, score
                # chunk-pairs 4t..4t+3 (keys AND query spans stay inside the
                # tile), the matching conv2 tiles and transposes, and the
                # previous tile's denominator matmuls.  The whole score loop
                # hides in the DMA shadow; ACT streams exp continuously.
                tr_batches = {
                    0: ([0, 1, 2, 3], [4, 5, 6, 7]),
                    1: ([8, 9, 10, 11], [12, 13, 14, 15]),
                    2: ([16, 17, 18, 19], [20, 21, 22, 23]),
                    3: ([24, 25, 26, 27], [28, 29, 30, 31], [32, 33]),
                }
                for t in range(4):
                    x0, x1 = xts[t]
                    for u in range(2):
                        pc = psA.tile([128, 512], F32, tag="psa", name=f"c1_{t}{u}")
                        usl = slice(512 * u, 512 * (u + 1))
                        nc.tensor.matmul(pc[:], w1t0[:], x0[:, usl],
                                         start=True, stop=False)
                        nc.tensor.matmul(pc[:], w1t1[:], x1[:, usl],
                                         start=False, stop=True)
                        o = 128 + 1024 * t + 512 * u
                        nc.vector.tensor_scalar(
                            out=hpad[:, o:o + 512], in0=pc[:],
                            scalar1=b1[:], scalar2=None, op0=ALU.add)
                    if t == 3:
                        for p in range(8, 12):
                            emit_dn(2 * p)
                            emit_dn(2 * p + 1)
                        emit_chain_newton(0)
                    for cp in range(4 * t, 4 * t + 4):
                        emit_scores(cp)
                    if t == 3:
                        emit_scores(16)
                        emit_chain_row(0)
                    emit_conv2(2 * t)
                    emit_conv2(2 * t + 1)
                    for batch in tr_batches[t]:
                        transpose_batch(batch)
                    if 1 <= t <= 2:
                        for p in range(4 * (t - 1), 4 * t):
                            emit_dn(2 * p)
                            emit_dn(2 * p + 1)

                for p in range(12, 16):
                    emit_dn(2 * p)
                    emit_dn(2 * p + 1)

                # ---- P3e: samples (pairs of g8 share one 2-bank psum tile),
                # gate+normalize, output conv; chain half 1 hides under the
                # first sample pair.
                sp_tiles = {}

                def emit_sample_pair(gp):
                    sp = psS.tile([128, 1024], F32, tag="psc", name=f"sp{gp}")
                    sp_tiles[gp] = sp
                    for a8 in range(8):
                        s8 = 8 * gp + a8
                        for j in range(3):
                            c = s8 + j
                            aa = 2 - j
                            nc.tensor.matmul(
                                sp[:, 128 * a8:128 * (a8 + 1)],
                                hT[:, 128 * c:128 * (c + 1)],
                                attnm[:, 512 * c + 128 * aa:512 * c + 128 * (aa + 1)],
                                start=(j == 0), stop=(j == 2))

                def emit_attr(g8):
                    sp = sp_tiles[g8 // 2]
                    gsl = slice(512 * g8, 512 * (g8 + 1))
                    nc.vector.tensor_tensor(
                        out=attr[:, gsl], in0=sp[:, 512 * (g8 % 2):512 * (g8 % 2 + 1)],
                        in1=Pg[:, gsl], op=ALU.mult)

                def emit_convout(g8):
                    gsl = slice(512 * g8, 512 * (g8 + 1))
                    for oc in range(2):
                        po = psA.tile([128, 512], F32, tag="psa",
                                      name=f"po{g8}_{oc}")
                        if oc == 1:
                            nc.tensor.matmul(po[:], boutrow[0:1, 128:256],
                                             ones1[0:1, :], start=True, stop=False)
                            nc.tensor.matmul(po[:], wot[:, 128:256], attr[:, gsl],
                                             start=False, stop=True)
                        else:
                            nc.tensor.matmul(po[:], wot[:, 0:128], attr[:, gsl],
                                             start=True, stop=True)
                        ot = otp.tile([128, 512], BF16, tag="ot")
                        if oc == 1:
                            nc.scalar.activation(ot[:], po[:], AF.Copy)
                        else:
                            nc.vector.tensor_scalar(out=ot[:], in0=po[:],
                                                    scalar1=bout[:, 0:1],
                                                    scalar2=None, op0=ALU.add)
                        nc.sync.dma_start(out_d[128 * oc:128 * (oc + 1), gsl],
                                          ot[:])

                emit_sample_pair(0)
                emit_pb_pgs(0)
                emit_pb_pgs(1)
                emit_dn(32)
                emit_dn(33)
                emit_chain_newton(1)
                emit_sample_pair(1)
                emit_chain_row(1)
                emit_attr(0)
                emit_convout(0)
                emit_attr(1)
                emit_convout(1)
                emit_pb_pgs(2)
                emit_pb_pgs(3)
                emit_sample_pair(2)
                emit_attr(2)
                emit_convout(2)
                emit_attr(3)
                emit_convout(3)
                emit_pb_pgs(4)
                emit_pb_pgs(5)
                emit_sample_pair(3)
                emit_attr(4)
                emit_convout(4)
                emit_attr(5)
                emit_convout(5)
                emit_pb_pgs(6)
                emit_pb_pgs(7)
                emit_attr(6)
                emit_convout(6)
                emit_attr(7)
                emit_convout(7)

    return nc


def _prep_inputs(x, W1, b1, W2, b2, Wout, bout):
    maskC, maskC2g, dcol = _build_mask_and_D()
    bf = ml_dtypes.bfloat16
    common = {
        "W1T": np.ascontiguousarray(W1.T).astype(bf),
        "b1": np.asarray(b1, np.float32).reshape(CH, 1),
        "W2T": np.ascontiguousarray(W2.T).astype(bf),
        "b2h": (0.5 * np.asarray(b2, np.float32)).reshape(CH, 1),
        "WoutT": np.ascontiguousarray(Wout.T).astype(bf),
        "bout2": np.ascontiguousarray(np.asarray(bout, np.float32).reshape(2, CH).T),
        "boutrow": np.asarray(bout, np.float32).reshape(1, CIN).astype(bf),
        "maskC": maskC,
        "maskC2g": maskC2g,
        "dcol": dcol,
        "ident": np.eye(128, dtype=np.float32).astype(bf),
        "onescol_c": np.ones((128, 1), np.float32).astype(bf),
        "ones1_c": np.ones((1, 512), np.float32).astype(bf),
    }
    in_maps = []
    for i in range(N):
        m = dict(common)
        m["x"] = np.ascontiguousarray(
            np.asarray(x[i], np.float32).reshape(CIN, HW)).astype(bf)
        in_maps.append(m)
    return in_maps


_CACHED = {}


def kernel(x, W1, b1, W2, b2, Wout, bout):
    if "nc" not in _CACHED:
        nc = build_nc()
        nc.finalize()
        _CACHED["nc"] = nc
    nc = _CACHED["nc"]
    in_maps = _prep_inputs(x, W1, b1, W2, b2, Wout, bout)
    res = run_bass_kernel_spmd(nc, in_maps, core_ids=list(range(N)))
    out = np.stack([res.results[i]["out"].reshape(CIN, H, W) for i in range(N)])
    return out.astype(np.float32)
